# revision 1
# baseline (speedup 1.0000x reference)
"""Causal single-head attention block on 8 TRN2 NeuronCores.

Problem: x[8, 2048, 1024] fp32; Wq/Wk/Wv [1024, 512]; bq/bk/bv [512].
  q = x@Wq + bq; k = x@Wk + bk; v = x@Wv + bv
  out = concat([x, softmax_causal(q k^T / sqrt(512)) @ v], axis=-1)

Sharding: data-parallel over batch — one batch element per core, no
collectives. Each core runs the identical program on its own slice.

Per-core algorithm (single batch element, S=2048, F=1024, D=V=512):
  Phase A/B (per 1024-row half of the sequence):
    - load x rows, DMA them straight back out as the passthrough half of
      the output, PE-transpose into xT[f,s] (contraction over features
      needs x feature-major on partitions)
    - qT[d,s] / kT[d,s] = Wq/Wk (stationary) @ xT, v[s,d] = xT
      (stationary) @ Wv, biases folded in on the PSUM->SBUF copy
  Phase C (flash-style causal attention, one 128-row q-tile at a time):
    - S strip [128 q x 512 k] = qT^T @ kT accumulated over 4 d-chunks
    - P = exp(S / sqrt(512)) with no max subtraction (logits of this
      distribution are bounded ~7, exp stays far from fp32 overflow);
      row sums accumulated via the activation accum_out
    - diagonal strips: multiplicative 0/1 causal mask (4 precomputed
      patterns), then a separate row-sum reduce
    - P chunks transposed on the PE, then read += P^T^T @ v chunk
    - read normalized by 1/rowsum, bv added, DMA'd to the output
  Matmuls run as float32r (full-rate fp32 PE mode).
"""

import numpy as np

import concourse.bass as bass
import concourse.bacc as bacc
import concourse.mybir as mybir
import concourse.tile as tile
from concourse.bass_utils import run_bass_kernel_spmd
from concourse.masks import make_identity

F32 = mybir.dt.float32
F32R = mybir.dt.float32r

B, S, F, D = 8, 2048, 1024, 512
NQ = S // 128          # 16 q-tiles
SCALE = 1.0 / np.sqrt(np.float32(D))

# matmul input dtype per stage (float32r = full-rate, float32 = 1/4 rate)
MM_DT = F32R


def _mm(x):
    return x.bitcast(MM_DT)


def build_program(reps=1):
    nc = bacc.Bacc("TRN2", target_bir_lowering=False, debug=False)

    x = nc.dram_tensor("x", [S, F], F32, kind="ExternalInput")
    Wq = nc.dram_tensor("Wq", [F, D], F32, kind="ExternalInput")
    bq = nc.dram_tensor("bq", [D], F32, kind="ExternalInput")
    Wk = nc.dram_tensor("Wk", [F, D], F32, kind="ExternalInput")
    bk = nc.dram_tensor("bk", [D], F32, kind="ExternalInput")
    Wv = nc.dram_tensor("Wv", [F, D], F32, kind="ExternalInput")
    bv = nc.dram_tensor("bv", [D], F32, kind="ExternalInput")
    out = nc.dram_tensor("out", [S, F + D], F32, kind="ExternalOutput")

    with tile.TileContext(nc) as tc:
        _emit(nc, tc, x, Wq, bq, Wk, bk, Wv, bv, out, reps=reps)
    nc.compile()
    return nc


def _emit(nc, tc, x, Wq, bq, Wk, bk, Wv, bv, out, reps=1):
    consts = tc.alloc_tile_pool(name="consts", bufs=1)
    qTp = tc.alloc_tile_pool(name="qTp", bufs=1)
    kTp = tc.alloc_tile_pool(name="kTp", bufs=1)
    vp = tc.alloc_tile_pool(name="vp", bufs=1)

    # persistent tensors
    qT = [consts_tile(qTp, [128, S], f"qT{d}") for d in range(4)]
    kT = [consts_tile(kTp, [128, S], f"kT{d}") for d in range(4)]
    v = [consts_tile(vp, [128, D], f"v{c}") for c in range(NQ)]

    ident = qTp.tile([128, 128], F32, tag="ident")
    make_identity(nc, ident[:, :])

    # per-partition bias columns for q/k (bias varies along the d axis,
    # which is the partition axis of qT/kT)
    bq_c, bk_c = [], []
    for d in range(4):
        for (src, lst, nm) in ((bq, bq_c, "bq"), (bk, bk_c, "bk")):
            t = consts.tile([128, 1], F32, tag=f"{nm}c{d}")
            nc.gpsimd.dma_start(out=t[:, :], in_=src[d * 128:(d + 1) * 128].rearrange("(p o) -> p o", o=1))
            lst.append(t)

    # bv broadcast across partitions (read bias varies along the free axis)
    bv_bc = consts.tile([128, D], F32, tag="bv_bc")
    nc.gpsimd.dma_start(out=bv_bc[:, :], in_=bv.ap().unsqueeze(0).partition_broadcast(128).rearrange("p o f -> p (o f)"))

    for _rep in range(reps):
        def attn_block(lo, hi):
            # ------------- phase C: causal attention --------------------------
            with tc.tile_pool(name=f"cmask{lo}", bufs=1) as cmp_, \
                 tc.tile_pool(name="psS", bufs=4, space="PSUM") as psSp, \
                 tc.tile_pool(name="psT", bufs=2, space="PSUM") as psTp, \
                 tc.tile_pool(name="psR", bufs=2, space="PSUM") as psRp, \
                 tc.tile_pool(name="Pp", bufs=3) as Pp, \
                 tc.tile_pool(name="tSp", bufs=3) as tSp, \
                 tc.tile_pool(name="omisc", bufs=2) as omisc:
                # 4 diagonal masks: mask[r][p, col] = 1 if 128*r + p >= col
                dmasks = []
                for r in range(4):
                    m = cmp_.tile([128, 512], F32, tag=f"dmask{r}", name=f"dmask{r}")
                    nc.gpsimd.memset(m[:, :], 1.0)
                    nc.gpsimd.affine_select(
                        out=m[:, :], in_=m[:, :],
                        compare_op=mybir.AluOpType.is_ge, fill=0.0,
                        base=128 * r, channel_multiplier=1, pattern=[[-1, 512]],
                    )
                    dmasks.append(m)

                order = list(range(hi - 1, lo - 1, -1))
                for i in order:
                    nk = i + 1                 # valid 128-wide k-chunks
                    ns = (nk + 3) // 4         # 512-wide strips
                    lacc = omisc.tile([128, 4], F32, tag="lacc", name="lacc")
                    psS = []
                    for s_ in range(ns):
                        w = min(512, nk * 128 - s_ * 512)  # valid width
                        ps = psSp.tile([128, 512], F32, tag="psS", name="psS")
                        psS.append(ps)
                        for d in range(4):
                            nc.tensor.matmul(
                                ps[:, 0:w],
                                lhsT=_mm(qT[d][:, i * 128:(i + 1) * 128]),
                                rhs=_mm(kT[d][:, s_ * 512:s_ * 512 + w]),
                                start=(d == 0), stop=(d == 3),
                            )
                    psR = psRp.tile([128, D], F32, tag="psR", name="psR")
                    prev = None  # (tS tile, chunk list) pending PV emission
                    for s_ in range(ns):
                        P = Pp.tile([128, 512], F32, tag="P", name="P")
                        if s_ == ns - 1:
                            w = nk * 128 - s_ * 512
                            nc.scalar.activation(
                                out=P[:, 0:w], in_=psS[s_][:, 0:w],
                                func=mybir.ActivationFunctionType.Exp, scale=float(SCALE))
                            nc.vector.tensor_tensor(
                                out=P[:, 0:w], in0=P[:, 0:w], in1=dmasks[i % 4][:, 0:w],
                                op=mybir.AluOpType.mult)
                            nc.vector.reduce_sum(
                                lacc[:, s_:s_ + 1], P[:, 0:w], axis=mybir.AxisListType.X)
                        else:
                            nc.scalar.activation(
                                out=P[:, :], in_=psS[s_][:, :],
                                func=mybir.ActivationFunctionType.Exp, scale=float(SCALE),
                                accum_out=lacc[:, s_:s_ + 1])
                        chunks = list(range(s_ * 4, min(s_ * 4 + 4, nk)))
                        pst = psTp.tile([128, 512], F32, tag="psT", name="psT")
                        for j, cch in enumerate(chunks):
                            nc.tensor.transpose(
                                pst[:, j * 128:(j + 1) * 128],
                                P[:, j * 128:(j + 1) * 128], ident[:, :])
                        tS = tSp.tile([128, 512], F32, tag="tS", name="tS")
                        nc.vector.tensor_copy(
                            out=tS[:, 0:len(chunks) * 128].bitcast(F32R),
                            in_=pst[:, 0:len(chunks) * 128])
                        if prev is not None:
                            for j, cch in enumerate(prev[1]):
                                nc.tensor.matmul(
                                    psR[:, :], lhsT=_mm(prev[0][:, j * 128:(j + 1) * 128]),
                                    rhs=_mm(v[cch][:, :]),
                                    start=(cch == 0), stop=(cch == nk - 1))
                        prev = (tS, chunks)
                    for j, cch in enumerate(prev[1]):
                        nc.tensor.matmul(
                            psR[:, :], lhsT=_mm(prev[0][:, j * 128:(j + 1) * 128]),
                            rhs=_mm(v[cch][:, :]),
                            start=(cch == 0), stop=(cch == nk - 1))
                    l = omisc.tile([128, 1], F32, tag="l", name="l")
                    nc.vector.reduce_sum(l[:, :], lacc[:, 0:ns], axis=mybir.AxisListType.X)
                    rl = omisc.tile([128, 1], F32, tag="rl", name="rl")
                    nc.vector.reciprocal(rl[:, :], l[:, :])
                    ot = omisc.tile([128, D], F32, tag="ot", name="ot")
                    nc.vector.tensor_scalar(
                        out=ot[:, :], in0=psR[:, :], scalar1=rl[:, :], scalar2=None,
                        op0=mybir.AluOpType.mult)
                    nc.vector.tensor_tensor(
                        out=ot[:, :], in0=ot[:, :], in1=bv_bc[:, :], op=mybir.AluOpType.add)
                    nc.sync.dma_start(out=out[i * 128:(i + 1) * 128, F:F + D], in_=ot[:, :])

        # ------------- phase A/B: x passthrough+transpose, projections ---
        for h in range(2):
            with tc.tile_pool(name=f"xTh{h}", bufs=1) as xTh_p, \
                 tc.tile_pool(name=f"wp{h}", bufs=2) as wp, \
                 tc.tile_pool(name=f"ws{h}", bufs=2) as wsp, \
                 tc.tile_pool(name=f"psX{h}", bufs=2, space="PSUM") as psX, \
                 tc.tile_pool(name=f"psQ{h}", bufs=4, space="PSUM") as psQ, \
                 tc.tile_pool(name=f"xstage{h}", bufs=2) as xstage:
                # xT for this half as one [128, 8*1024] tile; logical
                # layout [p, f, s] with f stride 1024
                xTb = xTh_p.tile([128, 8 * 1024], F32, tag="xTb", name="xTb")
                xTv = xTb[:, :].rearrange("p (f s) -> p f s", f=8)

                def load_round(W, wtag):
                    wt = []
                    for f in range(8):
                        t = wp.tile([128, D], F32, tag=f"w{f}", name=f"w{wtag}{f}")
                        ws = wsp.tile([128, D], F32, tag="ws", name="ws")
                        nc.sync.dma_start(out=ws[:, :], in_=W[f * 128:(f + 1) * 128, :])
                        nc.scalar.copy(out=t[:, :].bitcast(F32R), in_=ws[:, :])
                        wt.append(t)
                    return wt

                wtq = load_round(Wq, "q")
                wtv = load_round(Wv, "v")

                def qk_group(wt, d, st, bcols, dest):
                    ps = psQ.tile([128, 512], F32, tag="psq", name="psq")
                    for f in range(8):
                        nc.tensor.matmul(
                            ps[:, :],
                            lhsT=_mm(wt[f][:, d * 128:(d + 1) * 128]),
                            rhs=_mm(xTb[:, f * 1024 + st * 512:f * 1024 + st * 512 + 512]),
                            start=(f == 0), stop=(f == 7),
                        )
                    s0 = h * 1024 + st * 512
                    nc.vector.tensor_scalar_add(
                        out=dest[d][:, s0:s0 + 512].bitcast(F32R),
                        in0=ps[:, :], scalar1=bcols[d][:, :])

                for c8 in list(range(4)) + ["q_st0"] + list(range(4, 8)):
                    if c8 == "q_st0":
                        for d in range(4):
                            qk_group(wtq, d, 0, bq_c, qT)
                        continue
                    c = h * 8 + c8
                    xs = xstage.tile([128, F], F32, tag="xs", name="xs")
                    nc.sync.dma_start(out=xs[:, 0:512], in_=x[c * 128:(c + 1) * 128, 0:512])
                    nc.sync.dma_start(out=xs[:, 512:1024], in_=x[c * 128:(c + 1) * 128, 512:1024])
                    nc.sync.dma_start(out=out[c * 128:(c + 1) * 128, 0:F], in_=xs[:, :])
                    psx = psX.tile([128, 1024], F32, tag="psx", name="psx")
                    for f in range(8):
                        nc.tensor.transpose(
                            psx[:, f * 128:(f + 1) * 128],
                            xs[:, f * 128:(f + 1) * 128], ident[:, :])
                    # one strided copy: psx [p, f, 128] -> xT[p, f, c8*128:+128]
                    src = psx[:, :].rearrange("p (f s) -> p f s", f=8)
                    dst = xTv[:, :, c8 * 128:(c8 + 1) * 128].bitcast(F32R)
                    if c8 % 2 == 0:
                        nc.scalar.copy(out=dst, in_=src)
                    else:
                        nc.vector.tensor_copy(out=dst, in_=src)
                    # v projection for this chunk: lhsT is this chunk's xT
                    ps = psQ.tile([128, 512], F32, tag="psq", name="psq")
                    for f in range(8):
                        nc.tensor.matmul(
                            ps[:, :],
                            lhsT=_mm(xTb[:, f * 1024 + c8 * 128:f * 1024 + c8 * 128 + 128]),
                            rhs=_mm(wtv[f][:, :]),
                            start=(f == 0), stop=(f == 7),
                        )
                    nc.vector.tensor_tensor(
                        out=v[h * 8 + c8][:, :].bitcast(F32R), in0=ps[:, :],
                        in1=bv_bc[:, :], op=mybir.AluOpType.add)

                for d in range(4):
                    qk_group(wtq, d, 1, bq_c, qT)
                wtk = load_round(Wk, "k")
                for st in range(2):
                    for d in range(4):
                        qk_group(wtk, d, st, bk_c, kT)

            attn_block(8 * h, 8 * (h + 1))

    for p in (vp, kTp, qTp, consts):
        p.release()


def consts_tile(pool, shape, tag):
    return pool.tile(shape, F32, tag=tag, name=tag)


_NC_CACHE = None


def _get_program():
    global _NC_CACHE
    if _NC_CACHE is None:
        _NC_CACHE = build_program()
    return _NC_CACHE


def kernel(**inputs):
    nc = _get_program()
    arrs = {k: np.ascontiguousarray(np.asarray(v, dtype=np.float32))
            for k, v in inputs.items()}
    in_maps = []
    for b in range(B):
        m = {"x": arrs["x"][b]}
        for k in ("Wq", "bq", "Wk", "bk", "Wv", "bv"):
            m[k] = arrs[k]
        in_maps.append(m)
    res = run_bass_kernel_spmd(nc, in_maps, core_ids=list(range(B)))
    return np.stack([res.results[b]["out"] for b in range(B)], axis=0)



# revision 4
# speedup vs baseline: 1.2944x; 1.2944x over previous
"""Causal single-head attention block on 8 TRN2 NeuronCores — fp8 version.

Problem: x[8, 2048, 1024] fp32; Wq/Wk/Wv [1024, 512]; bq/bk/bv [512].
  q = x@Wq + bq; k = x@Wk + bk; v = x@Wv + bv
  out = concat([x, softmax_causal(q k^T / sqrt(512)) @ v], axis=-1)

Sharding: data-parallel over batch — one batch element per core, no
collectives.

Per-core algorithm (S=2048, F=1024, D=512), all matmuls in fp8(e4m3)
with DoubleRow perf mode (256-row contraction pairs, ~1.8x fp32r rate):

  Phase A: stage all of x in SBUF (one big DMA per 512-row group), DMA
    it back out as the passthrough columns, convert to fp8, PE-transpose
    into xT8[f, s].  Projections: qT8/kT8[d, s] = W^T x^T with W-pair
    stationary; v8[s, d] = x W with xT8-pair stationary.  Biases folded
    in on the PSUM->SBUF fp8-quantizing copies (bv lands inside v8, so
    the attention output P@v8/rowsum == read + bv exactly).

  Phase C (per 512-wide q block, flash-style over k chunks):
    S^T strip [128k x 512q] = kT8-pair^T @ qT8-pair (2 DR matmuls);
    P^T = exp(S/sqrt(512))/64 via the activation bias (P fits fp8's
    range; the 1/64 cancels in the normalization), fp8 out;
    diagonal chunks masked with one affine_select;
    PV: psR[q-tile] += P-pair^T @ v8-pair (DR), rowsums via a 1-column
    matmul against ones reusing the same stationary P-pair;
    normalize by 1/rowsum on DVE, DMA per-block to the output.
"""

import numpy as np

import concourse.bass as bass
import concourse.bacc as bacc
import concourse.mybir as mybir
import concourse.tile as tile
from concourse.bass_utils import run_bass_kernel_spmd
from concourse.masks import make_identity

F32 = mybir.dt.float32
BF16 = mybir.dt.bfloat16
F8 = mybir.dt.float8e4
DR = mybir.MatmulPerfMode.DoubleRow

B, S, F, D = 8, 2048, 1024, 512
NSC = S // 128         # 16 s-chunks
NFC = F // 128         # 8 f-chunks (4 DR pairs)
NDC = D // 128         # 4 d-chunks (2 DR pairs)
NBLK = 4               # q blocks of 512
QB = 4                 # q-tiles per block
SCALE = 1.0 / np.sqrt(np.float32(D))
PBIAS = float(-np.log(64.0))   # P scaled by 1/64 to fit fp8e4 range


def build_program(reps=1):
    nc = bacc.Bacc("TRN2", target_bir_lowering=False, debug=False)

    x = nc.dram_tensor("x", [S, F], F32, kind="ExternalInput")
    Wq = nc.dram_tensor("Wq", [F, D], F32, kind="ExternalInput")
    bq = nc.dram_tensor("bq", [D], F32, kind="ExternalInput")
    Wk = nc.dram_tensor("Wk", [F, D], F32, kind="ExternalInput")
    bk = nc.dram_tensor("bk", [D], F32, kind="ExternalInput")
    Wv = nc.dram_tensor("Wv", [F, D], F32, kind="ExternalInput")
    bv = nc.dram_tensor("bv", [D], F32, kind="ExternalInput")
    out = nc.dram_tensor("out", [S, F + D], F32, kind="ExternalOutput")

    with tile.TileContext(nc) as tc:
        _emit(nc, tc, x, Wq, bq, Wk, bk, Wv, bv, out, reps=reps)
    nc.compile()
    return nc


def _emit(nc, tc, x, Wq, bq, Wk, bk, Wv, bv, out, reps=1):
    consts = tc.alloc_tile_pool(name="consts", bufs=1)
    persist = tc.alloc_tile_pool(name="persist", bufs=1)

    # ---- constants (input-independent, outside the rep loop) ----
    identb = consts.tile([128, 128], BF16, tag="identb", name="identb")
    make_identity(nc, identb[:, :])
    ones8 = consts.tile([128, 2, 1], F8, tag="ones8", name="ones8")
    nc.gpsimd.memset(ones8[:, :, :], 1.0)
    pbias = consts.tile([128, 1], F32, tag="pbias", name="pbias")
    nc.gpsimd.memset(pbias[:, :], PBIAS)

    # per-partition bias columns for q/k (bias varies along d = partitions)
    bq_c, bk_c = [], []
    for dc in range(NDC):
        for (src, lst, nm) in ((bq, bq_c, "bq"), (bk, bk_c, "bk")):
            t = consts.tile([128, 1], F32, tag=f"{nm}c{dc}", name=f"{nm}c{dc}")
            nc.gpsimd.dma_start(
                out=t[:, :],
                in_=src[dc * 128:(dc + 1) * 128].rearrange("(p o) -> p o", o=1))
            lst.append(t)
    # bv broadcast across partitions (varies along free axis)
    bv_bc = consts.tile([128, D], F32, tag="bv_bc", name="bv_bc")
    nc.gpsimd.dma_start(
        out=bv_bc[:, :],
        in_=bv.ap().unsqueeze(0).partition_broadcast(128).rearrange("p o f -> p (o f)"))

    # ---- persistent buffers (rewritten every rep) ----
    xs = persist.tile([128, NSC, F], F32, tag="xs", name="xs")          # 64KB/p
    xT8 = persist.tile([128, NFC, S], F8, tag="xT8", name="xT8")        # 16KB/p
    qT8 = persist.tile([128, NDC, S], F8, tag="qT8", name="qT8")        # 8KB/p
    kT8 = persist.tile([128, NDC, S], F8, tag="kT8", name="kT8")        # 8KB/p
    v8 = persist.tile([128, NSC, D], F8, tag="v8", name="v8")           # 8KB/p
    w8 = {nm: persist.tile([128, NFC, D], F8, tag=f"w8{nm}", name=f"w8{nm}")
          for nm in ("q", "k", "v")}                                    # 12KB/p

    for _rep in range(reps):
        # =========== phase A: load, passthrough, transpose, project =========
        with tc.tile_pool(name="wstage", bufs=2) as wsp, \
             tc.tile_pool(name="x8p", bufs=2) as x8p, \
             tc.tile_pool(name="psx8", bufs=2, space="PSUM") as psx8p, \
             tc.tile_pool(name="psq", bufs=3, space="PSUM") as psqp:

            def load_w(Wsrc, nm):
                ws = wsp.tile([128, NFC, D], F32, tag="ws", name="ws")
                nc.sync.dma_start(
                    out=ws[:, :, :],
                    in_=Wsrc[:, :].rearrange("(c p) d -> p c d", p=128))
                nc.vector.tensor_copy(out=w8[nm][:, :, :], in_=ws[:, :, :])

            def load_x_group(g):
                nc.sync.dma_start(
                    out=xs[:, 4 * g:4 * g + 4, :],
                    in_=x[g * 512:(g + 1) * 512, :].rearrange(
                        "(c p) f -> p c f", p=128))
                # passthrough half of the output, straight back out
                nc.scalar.dma_start(
                    out=out[g * 512:(g + 1) * 512, 0:F].rearrange(
                        "(c p) f -> p c f", p=128),
                    in_=xs[:, 4 * g:4 * g + 4, :])

            # input DMA order on the sync ring: x g0 first (unblocks PE),
            # weights next (needed by first projections), rest of x after.
            load_x_group(0)
            load_w(Wq, "q")
            load_w(Wv, "v")
            load_x_group(1)
            load_w(Wk, "k")
            load_x_group(2)
            load_x_group(3)

            def transpose_chunk(sc):
                xbc = x8p.tile([128, F], BF16, tag="xbc", name="xbc")
                nc.gpsimd.tensor_copy(out=xbc[:, :], in_=xs[:, sc, :])
                pst = psx8p.tile([128, NFC, 128], BF16, tag="pst", name="pst")
                for j in range(NFC):
                    nc.tensor.transpose(
                        pst[:, j, :], xbc[:, j * 128:(j + 1) * 128], identb[:, :])
                nc.scalar.copy(
                    out=xT8[:, :, sc * 128:(sc + 1) * 128], in_=pst[:, :, :])

            def v_proj(sc):
                ps = psqp.tile([128, D], F32, tag="psq", name="psq")
                for fp in range(4):
                    nc.tensor.matmul(
                        ps[:, :],
                        lhsT=xT8[:, 2 * fp:2 * fp + 2, sc * 128:(sc + 1) * 128],
                        rhs=w8["v"][:, 2 * fp:2 * fp + 2, :],
                        start=(fp == 0), stop=(fp == 3), perf_mode=DR)
                nc.vector.tensor_tensor(
                    out=v8[:, sc, :], in0=ps[:, :], in1=bv_bc[:, :],
                    op=mybir.AluOpType.add)

            def qk_strip(nm, dest, bcols, st):
                for dc in range(NDC):
                    ps = psqp.tile([128, 512], F32, tag="psq", name="psq")
                    for fp in range(4):
                        nc.tensor.matmul(
                            ps[:, :],
                            lhsT=w8[nm][:, 2 * fp:2 * fp + 2, dc * 128:(dc + 1) * 128],
                            rhs=xT8[:, 2 * fp:2 * fp + 2, st * 512:(st + 1) * 512],
                            start=(fp == 0), stop=(fp == 3), perf_mode=DR)
                    nc.vector.tensor_scalar_add(
                        out=dest[:, dc, st * 512:(st + 1) * 512],
                        in0=ps[:, :], scalar1=bcols[dc][:, :])

            # software pipeline: PE order = T(sc), v(sc-1), ... , strips(g-1)
            prev_sc = None
            pending_strip = None
            for g in range(4):
                for sc in range(4 * g, 4 * g + 4):
                    transpose_chunk(sc)
                    if prev_sc is not None:
                        v_proj(prev_sc)
                    prev_sc = sc
                if pending_strip is not None:
                    st = pending_strip
                    qk_strip("q", qT8, bq_c, st)
                    qk_strip("k", kT8, bk_c, st)
                pending_strip = g
            v_proj(prev_sc)
            qk_strip("q", qT8, bq_c, 3)
            qk_strip("k", kT8, bk_c, 3)

        # =========== phase C: causal attention, 512-wide q blocks ===========
        with tc.tile_pool(name="psS", bufs=2, space="PSUM") as psSp, \
             tc.tile_pool(name="psR", bufs=1, space="PSUM") as psRp, \
             tc.tile_pool(name="psL", bufs=1, space="PSUM") as psLp, \
             tc.tile_pool(name="P8p", bufs=3) as P8pool, \
             tc.tile_pool(name="ostage", bufs=2) as ostp, \
             tc.tile_pool(name="omisc", bufs=4) as omisc:

            for Bk in range(NBLK):
                npairs = 2 * Bk + 2
                psR = [psRp.tile([128, D], F32, tag=f"psR{j}", name=f"psR{j}")
                       for j in range(QB)]
                psL = psLp.tile([128, QB], F32, tag="psL", name="psL")

                def emit_pv(P8p, pair):
                    first, last = (pair == 0), (pair == npairs - 1)
                    for j in range(QB):
                        nc.tensor.matmul(
                            psR[j][:, :],
                            lhsT=P8p[:, :, j * 128:(j + 1) * 128],
                            rhs=v8[:, 2 * pair:2 * pair + 2, :],
                            start=first, stop=last, perf_mode=DR,
                            skip_group_check=True)
                        # start=True only on the very first psL matmul of the
                        # block: a start marks the ENTIRE 2KB psum bank as
                        # pending-zero, so per-column starts would wipe the
                        # accumulation of previously-started columns.
                        nc.tensor.matmul(
                            psL[:, j:j + 1],
                            lhsT=P8p[:, :, j * 128:(j + 1) * 128],
                            rhs=ones8[:, :, :],
                            start=(first and j == 0), stop=last, perf_mode=DR,
                            skip_group_check=True)

                prev_pair = None
                for pair in range(npairs):
                    P8p = P8pool.tile([128, 2, 512], F8, tag="P8", name="P8")
                    for half in range(2):
                        kc = 2 * pair + half
                        psSt = psSp.tile([128, 512], F32, tag="psS", name="psS")
                        for dp in range(2):
                            nc.tensor.matmul(
                                psSt[:, :],
                                lhsT=kT8[:, 2 * dp:2 * dp + 2, kc * 128:(kc + 1) * 128],
                                rhs=qT8[:, 2 * dp:2 * dp + 2, Bk * 512:(Bk + 1) * 512],
                                start=(dp == 0), stop=(dp == 1), perf_mode=DR)
                        nc.scalar.activation(
                            out=P8p[:, half, :], in_=psSt[:, :],
                            func=mybir.ActivationFunctionType.Exp,
                            scale=float(SCALE), bias=pbias[:, :])
                        if kc >= 4 * Bk:
                            # causal: keep q >= k, i.e. c - p + 512*Bk - 128*kc >= 0
                            nc.gpsimd.affine_select(
                                out=P8p[:, half, :], in_=P8p[:, half, :],
                                compare_op=mybir.AluOpType.is_ge, fill=0.0,
                                base=512 * Bk - 128 * kc,
                                channel_multiplier=-1, pattern=[[1, 512]])
                    if prev_pair is not None:
                        emit_pv(*prev_pair)
                    prev_pair = (P8p, pair)
                emit_pv(*prev_pair)

                ot = ostp.tile([128, QB, D], F32, tag="ot", name="ot")
                for j in range(QB):
                    rl = omisc.tile([128, 1], F32, tag="rl", name="rl")
                    nc.vector.reciprocal(rl[:, :], psL[:, j:j + 1])
                    nc.vector.tensor_scalar(
                        out=ot[:, j, :], in0=psR[j][:, :], scalar1=rl[:, :],
                        scalar2=None, op0=mybir.AluOpType.mult)
                nc.scalar.dma_start(
                    out=out[Bk * 512:(Bk + 1) * 512, F:F + D].rearrange(
                        "(c p) d -> p c d", p=128),
                    in_=ot[:, :, :])

    for p in (persist, consts):
        p.release()


_NC_CACHE = None


def _get_program():
    global _NC_CACHE
    if _NC_CACHE is None:
        _NC_CACHE = build_program()
    return _NC_CACHE


def kernel(**inputs):
    nc = _get_program()
    arrs = {k: np.ascontiguousarray(np.asarray(v, dtype=np.float32))
            for k, v in inputs.items()}
    in_maps = []
    for b in range(B):
        m = {"x": arrs["x"][b]}
        for k in ("Wq", "bq", "Wk", "bk", "Wv", "bv"):
            m[k] = arrs[k]
        in_maps.append(m)
    res = run_bass_kernel_spmd(nc, in_maps, core_ids=list(range(B)))
    return np.stack([res.results[b]["out"] for b in range(B)], axis=0)
